# revision 3
# baseline (speedup 1.0000x reference)
"""Causal self-attention (RoPE) Trainium2 kernel, 8-core SPMD (bf16).

Sharding: core c -> (batch b = c//2, head-group g = c%2). Each core computes
its batch's attention output restricted to its 8 heads, then applies the
corresponding 512 rows of Wo^T; the host sums the two head-group partials
per batch (Megatron-style row-parallel output projection).

v2 changes vs baseline:
  - all matmul operands bf16 (halves SBUF + DMA, bf16 runs 1 cyc/row at any
    free size so the f32r N<256 4x penalty is gone)
  - x loaded once into a resident SBUF tile (baseline streamed it twice)
  - all weights resident; DMAs split per o-chunk for fast pipeline start
  - output projection DMAs directly from PSUM (no staging copy)
  - diag-mask mul merged across the head pair (one Pool op per j block)
  - RoPE raw copy moved to DVE (keeps ScalarE free for exp)

Device layout is fully transposed (features on partitions):
  XS  (128, 8, S)                       resident, bf16
  QT/KT = Wg @ X^T  (128, 4, S)         RoPE applied via PE rotation matmul
  scoresT = K @ Q^T (s2 parts, s1 free) exp on ScalarE, causal blocks only
  OT_aug = [V | 1]^T @ P^T (65, s1)     row 64 = softmax denominator
  partial = OT^T @ Wo_g^T               accumulated over 4 K-tiles on PE
"""

import math

import numpy as np

B, S, DIM = 4, 2048, 1024
NUM_HEADS = 16
HEAD_DIM = 64
ROPE_BASE = 10000.0
N_CORES = 8
HG = 8          # heads per core (head-group)
O = HG * HEAD_DIM  # 512 per-core projection width
C1 = 1024       # phase-2 s1 chunk width

_NC = {}  # cached compiled Bass programs, keyed by reps


def _rope_tables():
    inv_freq = 1.0 / (ROPE_BASE ** (np.arange(0, HEAD_DIM, 2, dtype=np.float64) / HEAD_DIM))
    t = np.arange(S, dtype=np.float64)
    freqs = np.einsum("i,j->ij", t, inv_freq)          # (S, 32)
    emb = np.concatenate([freqs, freqs], axis=-1)      # (S, 64)
    cos = np.cos(emb).astype(np.float32)
    sin = np.sin(emb).astype(np.float32)
    # transposed + tiled to 128 partitions (2 heads per 128-row tile)
    cosT = np.tile(cos.T, (2, 1))                      # (128, S)
    sinT = np.tile(sin.T, (2, 1))
    return cosT, sinT


def _rot_matrix():
    # rotate_half as a matrix: out[d] = -q[d+32] (d<32), q[d-32] (d>=32)
    r = np.zeros((HEAD_DIM, HEAD_DIM), dtype=np.float32)
    for d in range(32):
        r[d, d + 32] = -1.0
        r[d + 32, d] = 1.0
    r128 = np.zeros((128, 128), dtype=np.float32)
    r128[:64, :64] = r
    r128[64:, 64:] = r
    return r128.T.copy()  # lhsT for out = R @ q


def _build_nc(reps=1):
    from contextlib import ExitStack

    import concourse.mybir as mybir
    import concourse.tile as tile
    from concourse import bacc

    f32 = mybir.dt.float32
    bf16 = mybir.dt.bfloat16

    nc = bacc.Bacc("TRN2", target_bir_lowering=False, debug=False,
                   num_devices=N_CORES)

    xT = nc.declare_dram_parameter("xT", [DIM, S], bf16, isOutput=False)
    wqT = nc.declare_dram_parameter("wqT", [DIM, O], bf16, isOutput=False)
    wkT = nc.declare_dram_parameter("wkT", [DIM, O], bf16, isOutput=False)
    wvT = nc.declare_dram_parameter("wvT", [DIM, O], bf16, isOutput=False)
    woT = nc.declare_dram_parameter("woT", [O, DIM], bf16, isOutput=False)
    cosT = nc.declare_dram_parameter("cosT", [128, S], bf16, isOutput=False)
    sinT = nc.declare_dram_parameter("sinT", [128, S], bf16, isOutput=False)
    rT = nc.declare_dram_parameter("rT", [128, 128], bf16, isOutput=False)
    dmask = nc.declare_dram_parameter("dmask", [128, 2, 128], bf16,
                                      isOutput=False)
    out = nc.declare_dram_parameter("out", [S, DIM], f32, isOutput=True)

    xT3 = xT.ap().rearrange("(o p) s -> p o s", p=128)      # (128, 8, S)
    wq3 = wqT.ap().rearrange("(o p) f -> p o f", p=128)     # (128, 8, 512)
    wk3 = wkT.ap().rearrange("(o p) f -> p o f", p=128)
    wv3 = wvT.ap().rearrange("(o p) f -> p o f", p=128)
    wo3 = woT.ap().rearrange("(o p) f -> p o f", p=128)     # (128, 4, 1024)

    with tile.TileContext(nc) as tc, ExitStack() as prog:
        for _ in range(reps):
            with ExitStack() as top:
                _emit_body(nc, tc, top, mybir, f32, bf16,
                           xT3, wq3, wk3, wv3, wo3, cosT, sinT, rT, dmask,
                           out)

    nc.compile()
    return nc


def _emit_body(nc, tc, top, mybir, f32, bf16,
               xT3, wq3, wk3, wv3, wo3, cosT, sinT, rT, dmask, out):
    from contextlib import ExitStack

    const = top.enter_context(tc.tile_pool(name="const", bufs=1))
    XS = const.tile([128, 8, S], bf16)          # resident x, transposed
    WQ = const.tile([128, 8, O], bf16)
    WK = const.tile([128, 8, O], bf16)
    WV = const.tile([128, 8, O], bf16)
    WO = const.tile([128, 4, DIM], bf16)
    cos_sb = const.tile([128, S], bf16)
    sin_sb = const.tile([128, S], bf16)
    rt_sb = const.tile([128, 128], bf16)
    dm_sb = const.tile([128, 2, 128], bf16)

    ot_pool = top.enter_context(tc.tile_pool(name="ot", bufs=1))
    OT = ot_pool.tile([128, 4, S], bf16)  # normalized attn out, transposed

    CW = 512  # attention s1 chunk width (1 PSUM bank per score tile)

    with ExitStack() as mid:
        qk_pool = mid.enter_context(tc.tile_pool(name="qk", bufs=1))
        QT = qk_pool.tile([128, 4, S], bf16)
        KT = qk_pool.tile([128, 4, S], bf16)
        VA = qk_pool.tile([128, 16, 520], bf16)  # [V(64) | ones] per head
        epn = mid.enter_context(tc.tile_pool(name="epn", bufs=5))
        bp = mid.enter_context(tc.tile_pool(name="bp", bufs=2))
        rp = mid.enter_context(tc.tile_pool(name="rp", bufs=2))
        stg = mid.enter_context(tc.tile_pool(name="stg", bufs=3))
        tp = mid.enter_context(tc.tile_pool(name="tp", bufs=4))
        rawp = mid.enter_context(tc.tile_pool(name="rawp", bufs=4))
        # PSUM: ps1 2x1 + scn 2x2 + otn 2x1 = 8 banks, all pools live for
        # the whole kernel -> no phase-boundary bank-reuse serialization.
        ps1 = mid.enter_context(tc.tile_pool(name="ps1", bufs=2,
                                             space="PSUM"))
        scn = mid.enter_context(tc.tile_pool(name="scn", bufs=2,
                                             space="PSUM"))
        otn = mid.enter_context(tc.tile_pool(name="otn", bufs=2,
                                             space="PSUM"))

        def attn_chunk(ot, cw_lo):
            """Attention for head-pair `ot`, query cols [cw_lo, cw_lo+CW).

            Scores/exp/PV software-pipelined with PV lagging 2 j-blocks so
            PV matmuls never head-block the PE queue on a pending exp.
            """
            j_hi = (cw_lo + CW) // 128
            otps = {}
            for hb in (0, 64):
                otps[hb] = otn.tile([65, CW], f32, tag="otps",
                                    name=f"otps{hb}")

            def emit_pv(j, l0, et3):
                # diag blocks: split so only the masked 128 cols wait on
                # the mask op (range-precise deps let the rest go early)
                diag = 128 * j - cw_lo >= 0
                for hi, hb in enumerate((0, 64)):
                    h = 2 * ot + hi
                    mid = l0 + 128 if (diag and l0 + 128 < CW) else CW
                    nc.tensor.matmul(
                        otps[hb][:, l0:mid],
                        VA[:, j, h * 65:(h + 1) * 65],
                        et3[:, hi, l0:mid],
                        start=(j == 0), stop=(j == j_hi - 1))
                    if mid < CW:
                        nc.tensor.matmul(
                            otps[hb][:, mid:CW],
                            VA[:, j, h * 65:(h + 1) * 65],
                            et3[:, hi, mid:CW],
                            start=(j == 0), stop=(j == j_hi - 1))

            pending = []
            for j in range(j_hi):      # s2 blocks of 128
                l0 = max(0, 128 * j - cw_lo)
                et3 = epn.tile([128, 2, CW], bf16, tag="e", name="et")
                sc_ps = scn.tile([128, 2, CW], f32, tag="sc", name="sc_ps")
                for hi, hb in enumerate((0, 64)):
                    nc.tensor.matmul(
                        sc_ps[:, hi, l0:CW],
                        KT[hb:hb + 64, ot, j * 128:(j + 1) * 128],
                        QT[hb:hb + 64, ot, cw_lo + l0:cw_lo + CW],
                        start=True, stop=True)
                nc.scalar.activation(
                    et3[:, :, l0:CW], sc_ps[:, :, l0:CW],
                    mybir.ActivationFunctionType.Exp,
                    scale=1.0 / math.sqrt(HEAD_DIM))
                dl = 128 * j - cw_lo
                if dl >= 0:
                    nc.vector.tensor_mul(
                        et3[:, :, dl:dl + 128],
                        et3[:, :, dl:dl + 128], dm_sb[:])
                pending.append((j, l0, et3))
                if len(pending) > 2:
                    emit_pv(*pending.pop(0))
            for p in pending:
                emit_pv(*p)
            for hb in (0, 64):
                rec = rp.tile([1, CW], f32, tag="rec", name="rec")
                nc.vector.reciprocal(rec[:], otps[hb][64:65, :])
                bc = bp.tile([64, CW], f32, tag="bc", name="bc")
                nc.gpsimd.partition_broadcast(bc[:], rec[:])
                nc.vector.tensor_mul(
                    OT[hb:hb + 64, ot, cw_lo:cw_lo + CW],
                    otps[hb][0:64, :], bc[:])

        def proj_blocks(sbs):
            # ---- output projection, one 128-row s1 block per sb ----
            for sb in sbs:
                st = stg.tile([128, DIM], f32, tag="st", name="st")
                pj = scn.tile([128, 2, 512], f32, tag="sc", name="pj")
                for half in range(2):
                    for kt in range(4):
                        nc.tensor.matmul(
                            pj[:, half, :],
                            OT[:, kt, sb * 128:(sb + 1) * 128],
                            WO[:, kt, half * 512:(half + 1) * 512],
                            start=(kt == 0), stop=(kt == 3))
                nc.vector.tensor_copy(
                    st[:].rearrange("p (h f) -> p h f", h=2), pj[:])
                nc.sync.dma_start(
                    out.ap()[sb * 128:(sb + 1) * 128, :], st[:])

        def qk_ot(ot, scs=(0, 1, 2, 3)):
            for sc in scs:
                sl = slice(sc * 512, (sc + 1) * 512)
                for wsb, dest in ((WK, KT), (WQ, QT)):
                    acc = ps1.tile([128, 512], f32, tag="ps1", name="acc")
                    for kt in range(8):
                        nc.tensor.matmul(
                            acc[:],
                            wsb[:, kt, ot * 128:(ot + 1) * 128],
                            XS[:, kt, sl],
                            start=(kt == 0), stop=(kt == 7))
                    raw = rawp.tile([128, 512], bf16, tag="raw", name="raw")
                    nc.vector.tensor_copy(raw[:], acc[:])
                    rot = ps1.tile([128, 512], f32, tag="ps1", name="rot")
                    nc.tensor.matmul(rot[:], rt_sb[:], raw[:],
                                     start=True, stop=True)
                    t1 = tp.tile([128, 512], bf16, tag="t", name="t1")
                    nc.vector.tensor_mul(t1[:], raw[:], cos_sb[:, sl])
                    t2 = tp.tile([128, 512], bf16, tag="t", name="t2")
                    nc.vector.tensor_mul(t2[:], rot[:], sin_sb[:, sl])
                    nc.vector.tensor_add(dest[:, ot, sl], t1[:], t2[:])

        # --- input DMAs.  x + K/Q weights on sync (x first so the v
        # pipeline starts immediately); V weights on scalar; tables on
        # gpsimd (software DGE, separate path).
        nc.sync.dma_start(XS[:, :, 0:256], xT3[:, :, 0:256])
        for o in range(8):
            nc.sync.dma_start(WV[:, o, :], wv3[:, o, :])
        for sc2 in range(1, 8):
            sl = slice(sc2 * 256, (sc2 + 1) * 256)
            nc.sync.dma_start(XS[:, :, sl], xT3[:, :, sl])
        for o in range(8):
            nc.sync.dma_start(WK[:, o, :], wk3[:, o, :])
        for o in range(8):
            nc.sync.dma_start(WQ[:, o, :], wq3[:, o, :])
        nc.gpsimd.dma_start(rt_sb[:], rT.ap())
        nc.gpsimd.dma_start(cos_sb[:], cosT.ap())
        nc.gpsimd.dma_start(sin_sb[:], sinT.ap())
        nc.gpsimd.dma_start(dm_sb[:], dmask.ap())
        nc.gpsimd.dma_start(WO[:], wo3)

        # ones column of VA (written once; disjoint from V copies)
        va4 = VA[:].rearrange("p t (h c) -> p t h c", c=65)
        nc.vector.tensor_scalar(
            va4[:, :, :, 64:65], va4[:, :, :, 64:65],
            0.0, 1.0, mybir.AluOpType.mult, mybir.AluOpType.add)

        # --- V projection (s2 blocks of 128; tail blocks deferred into ---
        # --- the post-qk region as PE-dense filler)                     ---
        def v_blocks(s2ts, copy_eng):
            for s2t in s2ts:
                acc = ps1.tile([128, O], f32, tag="ps1", name="vacc")
                for kt in range(8):
                    nc.tensor.matmul(
                        acc[:],
                        XS[:, kt, s2t * 128:(s2t + 1) * 128],
                        WV[:, kt, :],
                        start=(kt == 0), stop=(kt == 7))
                vsl = VA[:, s2t, :].rearrange("p (h c) -> p h c", c=65)
                copy_eng(
                    vsl[:, :, 0:64],
                    acc[:].rearrange("p (h c) -> p h c", c=64))

        v_blocks(range(12), nc.scalar.copy)

        # --- interleave: PE-dense qk/proj segments alternate with ---
        # --- ACT-dense attention chunks                            ---
        qk_ot(0)
        attn_chunk(0, 0 * CW)
        qk_ot(1)
        attn_chunk(0, 1 * CW)
        attn_chunk(1, 0 * CW)
        qk_ot(2)
        attn_chunk(0, 2 * CW)
        attn_chunk(1, 1 * CW)
        attn_chunk(2, 0 * CW)
        qk_ot(3)
        attn_chunk(1, 2 * CW)
        v_blocks((12, 13), nc.vector.tensor_copy)
        attn_chunk(2, 1 * CW)
        v_blocks((14, 15), nc.vector.tensor_copy)
        attn_chunk(0, 3 * CW)
        attn_chunk(3, 0 * CW)
        attn_chunk(1, 3 * CW)
        proj_blocks([0, 1, 2, 3])
        attn_chunk(2, 2 * CW)
        attn_chunk(3, 1 * CW)
        attn_chunk(2, 3 * CW)
        proj_blocks([4, 5, 6, 7])
        attn_chunk(3, 2 * CW)
        attn_chunk(3, 3 * CW)
        proj_blocks([8, 9, 10, 11])
        proj_blocks([12, 13, 14, 15])


def _get_nc(reps=1):
    if reps not in _NC:
        _NC[reps] = _build_nc(reps)
    return _NC[reps]


def make_in_maps(x, Wq, Wk, Wv, Wo):
    import ml_dtypes
    bf16 = ml_dtypes.bfloat16
    cosT, sinT = _rope_tables()
    rT = _rot_matrix()
    # keep where s2 <= s1 in (s2, s1) indexing -> upper-tri incl diag,
    # duplicated for the merged head-pair mul
    dm1 = np.triu(np.ones((128, 128), dtype=np.float32))
    dm = np.stack([dm1, dm1], axis=1).astype(bf16)      # (128, 2, 128)
    in_maps = []
    for c in range(N_CORES):
        b, g = c // 2, c % 2
        rows = slice(g * O, (g + 1) * O)
        in_maps.append({
            "xT": np.ascontiguousarray(x[b].T).astype(bf16),
            "wqT": np.ascontiguousarray(Wq[rows, :].T).astype(bf16),
            "wkT": np.ascontiguousarray(Wk[rows, :].T).astype(bf16),
            "wvT": np.ascontiguousarray(Wv[rows, :].T).astype(bf16),
            "woT": np.ascontiguousarray(Wo[:, rows].T).astype(bf16),
            "cosT": cosT.astype(bf16), "sinT": sinT.astype(bf16),
            "rT": rT.astype(bf16), "dmask": dm,
        })
    return in_maps


def _numpy_fallback(x, Wq, Wk, Wv, Wo, mask):
    cosT, sinT = _rope_tables()
    cos, sin = cosT[:64].T, sinT[:64].T                      # (S, 64)
    xq = x @ Wq.T
    xk = x @ Wk.T
    xv = x @ Wv.T

    def heads(t):
        return t.reshape(B, S, NUM_HEADS, HEAD_DIM).transpose(0, 2, 1, 3)

    q, k, v = heads(xq), heads(xk), heads(xv)

    def rot(t):
        return np.concatenate([-t[..., 32:], t[..., :32]], axis=-1)

    q = q * cos + rot(q) * sin
    k = k * cos + rot(k) * sin
    sc = np.einsum("bhsd,bhtd->bhst", q, k) / math.sqrt(HEAD_DIM)
    sc = np.where(mask[None, None] == 0, -np.inf, sc)
    sc = sc - sc.max(axis=-1, keepdims=True)
    e = np.exp(sc)
    p = e / e.sum(axis=-1, keepdims=True)
    o = np.einsum("bhst,bhtd->bhsd", p, v)
    o = o.transpose(0, 2, 1, 3).reshape(B, S, DIM)
    return (o @ Wo.T).astype(np.float32)


def kernel(x, Wq, Wk, Wv, Wo, mask):
    x = np.asarray(x)
    mask = np.asarray(mask)
    causal = bool(
        np.array_equal(np.asarray(mask, dtype=np.int64),
                       np.tril(np.ones((S, S), dtype=np.int64))))
    if not causal:
        return _numpy_fallback(
            np.asarray(x, np.float32), np.asarray(Wq, np.float32),
            np.asarray(Wk, np.float32), np.asarray(Wv, np.float32),
            np.asarray(Wo, np.float32), mask)

    from concourse.bass_utils import run_bass_kernel_spmd

    nc = _get_nc()
    in_maps = make_in_maps(x, Wq, Wk, Wv, Wo)
    res = run_bass_kernel_spmd(nc, in_maps, list(range(N_CORES)))
    out = np.empty((B, S, DIM), dtype=np.float32)
    for b in range(B):
        out[b] = res.results[2 * b]["out"] + res.results[2 * b + 1]["out"]
    return out


# revision 6
# speedup vs baseline: 1.1542x; 1.1542x over previous
"""Causal self-attention (RoPE) Trainium2 kernel, 8-core SPMD (bf16).

Sharding: core c -> (batch b = c//2, head-group g = c%2). Each core computes
its batch's attention output restricted to its 8 heads, then applies the
corresponding 512 rows of Wo^T; the host sums the two head-group partials
per batch (Megatron-style row-parallel output projection).

v2 changes vs baseline:
  - all matmul operands bf16 (halves SBUF + DMA, bf16 runs 1 cyc/row at any
    free size so the f32r N<256 4x penalty is gone)
  - x loaded once into a resident SBUF tile (baseline streamed it twice)
  - all weights resident; DMAs split per o-chunk for fast pipeline start
  - diag-mask mul merged across the head pair, applied on DVE with the
    diagonal PV split so only the masked 128 cols wait on it
  - uniform 512-wide attention chunks with a static 8-bank PSUM split so
    projections, attention and output-projection all pipeline with no
    phase barriers; V-projection tail deferred as PE filler

Device layout is fully transposed (features on partitions):
  XS  (128, 8, S)                       resident, bf16
  QT/KT = Wg @ X^T  (128, 4, S)         RoPE applied via PE rotation matmul
  scoresT = K @ Q^T (s2 parts, s1 free) exp on ScalarE, causal blocks only
  OT_aug = [V | 1]^T @ P^T (65, s1)     row 64 = softmax denominator
  partial = OT^T @ Wo_g^T               accumulated over 4 K-tiles on PE
"""

import math

import numpy as np

B, S, DIM = 4, 2048, 1024
NUM_HEADS = 16
HEAD_DIM = 64
ROPE_BASE = 10000.0
N_CORES = 8
HG = 8          # heads per core (head-group)
O = HG * HEAD_DIM  # 512 per-core projection width
C1 = 1024       # phase-2 s1 chunk width

_NC = {}  # cached compiled Bass programs, keyed by reps


def _rope_tables():
    inv_freq = 1.0 / (ROPE_BASE ** (np.arange(0, HEAD_DIM, 2, dtype=np.float64) / HEAD_DIM))
    t = np.arange(S, dtype=np.float64)
    freqs = np.einsum("i,j->ij", t, inv_freq)          # (S, 32)
    emb = np.concatenate([freqs, freqs], axis=-1)      # (S, 64)
    cos = np.cos(emb).astype(np.float32)
    sin = np.sin(emb).astype(np.float32)
    # transposed + tiled to 128 partitions (2 heads per 128-row tile)
    cosT = np.tile(cos.T, (2, 1))                      # (128, S)
    sinT = np.tile(sin.T, (2, 1))
    return cosT, sinT


def _rot_matrix():
    # rotate_half as a matrix: out[d] = -q[d+32] (d<32), q[d-32] (d>=32)
    r = np.zeros((HEAD_DIM, HEAD_DIM), dtype=np.float32)
    for d in range(32):
        r[d, d + 32] = -1.0
        r[d + 32, d] = 1.0
    r128 = np.zeros((128, 128), dtype=np.float32)
    r128[:64, :64] = r
    r128[64:, 64:] = r
    return r128.T.copy()  # lhsT for out = R @ q


def _build_nc(reps=1):
    from contextlib import ExitStack

    import concourse.mybir as mybir
    import concourse.tile as tile
    from concourse import bacc

    f32 = mybir.dt.float32
    bf16 = mybir.dt.bfloat16

    nc = bacc.Bacc("TRN2", target_bir_lowering=False, debug=False,
                   num_devices=N_CORES)

    xT = nc.declare_dram_parameter("xT", [DIM, S], bf16, isOutput=False)
    wqT = nc.declare_dram_parameter("wqT", [DIM, O], bf16, isOutput=False)
    wkT = nc.declare_dram_parameter("wkT", [DIM, O], bf16, isOutput=False)
    wvT = nc.declare_dram_parameter("wvT", [DIM, O], bf16, isOutput=False)
    woT = nc.declare_dram_parameter("woT", [O, DIM], bf16, isOutput=False)
    cosT = nc.declare_dram_parameter("cosT", [128, S], bf16, isOutput=False)
    sinT = nc.declare_dram_parameter("sinT", [128, S], bf16, isOutput=False)
    rT = nc.declare_dram_parameter("rT", [128, 128], bf16, isOutput=False)
    dmask = nc.declare_dram_parameter("dmask", [128, 2, 128], bf16,
                                      isOutput=False)
    out = nc.declare_dram_parameter("out", [S, DIM], f32, isOutput=True)

    xT3 = xT.ap().rearrange("(o p) s -> p o s", p=128)      # (128, 8, S)
    wq3 = wqT.ap().rearrange("(o p) f -> p o f", p=128)     # (128, 8, 512)
    wk3 = wkT.ap().rearrange("(o p) f -> p o f", p=128)
    wv3 = wvT.ap().rearrange("(o p) f -> p o f", p=128)
    wo3 = woT.ap().rearrange("(o p) f -> p o f", p=128)     # (128, 4, 1024)

    with tile.TileContext(nc) as tc, ExitStack() as prog:
        for _ in range(reps):
            with ExitStack() as top:
                _emit_body(nc, tc, top, mybir, f32, bf16,
                           xT3, wq3, wk3, wv3, wo3, cosT, sinT, rT, dmask,
                           out)

    nc.compile()
    return nc


def _emit_body(nc, tc, top, mybir, f32, bf16,
               xT3, wq3, wk3, wv3, wo3, cosT, sinT, rT, dmask, out):
    from contextlib import ExitStack

    const = top.enter_context(tc.tile_pool(name="const", bufs=1))
    XS = const.tile([128, 8, S], bf16)          # resident x, transposed
    WQ = const.tile([128, 8, O], bf16)
    WK = const.tile([128, 8, O], bf16)
    WV = const.tile([128, 8, O], bf16)
    WO = const.tile([128, 4, DIM], bf16)
    cos_sb = const.tile([128, S], bf16)
    sin_sb = const.tile([128, S], bf16)
    rt_sb = const.tile([128, 128], bf16)
    dm_sb = const.tile([128, 2, 128], bf16)

    ot_pool = top.enter_context(tc.tile_pool(name="ot", bufs=1))
    OT = ot_pool.tile([128, 4, S], bf16)  # normalized attn out, transposed

    CW = 512  # attention s1 chunk width (1 PSUM bank per score tile)

    with ExitStack() as mid:
        qk_pool = mid.enter_context(tc.tile_pool(name="qk", bufs=1))
        QT = qk_pool.tile([128, 4, S], bf16)
        KT = qk_pool.tile([128, 4, S], bf16)
        VA = qk_pool.tile([128, 16, 520], bf16)  # [V(64) | ones] per head
        epn = mid.enter_context(tc.tile_pool(name="epn", bufs=5))
        bp = mid.enter_context(tc.tile_pool(name="bp", bufs=2))
        rp = mid.enter_context(tc.tile_pool(name="rp", bufs=2))
        stg = mid.enter_context(tc.tile_pool(name="stg", bufs=3))
        tp = mid.enter_context(tc.tile_pool(name="tp", bufs=4))
        rawp = mid.enter_context(tc.tile_pool(name="rawp", bufs=4))
        # PSUM: ps1 2x1 + scn 2x2 + otn 2x1 = 8 banks, all pools live for
        # the whole kernel -> no phase-boundary bank-reuse serialization.
        ps1 = mid.enter_context(tc.tile_pool(name="ps1", bufs=2,
                                             space="PSUM"))
        scn = mid.enter_context(tc.tile_pool(name="scn", bufs=2,
                                             space="PSUM"))
        otn = mid.enter_context(tc.tile_pool(name="otn", bufs=2,
                                             space="PSUM"))

        def attn_chunk(ot, cw_lo):
            """Attention for head-pair `ot`, query cols [cw_lo, cw_lo+CW).

            Scores/exp/PV software-pipelined with PV lagging 2 j-blocks so
            PV matmuls never head-block the PE queue on a pending exp.
            """
            j_hi = (cw_lo + CW) // 128
            otps = {}
            for hb in (0, 64):
                otps[hb] = otn.tile([65, CW], f32, tag="otps",
                                    name=f"otps{hb}")

            def emit_pv(j, l0, et3):
                # diag blocks: split so only the masked 128 cols wait on
                # the mask op (range-precise deps let the rest go early)
                diag = 128 * j - cw_lo >= 0
                for hi, hb in enumerate((0, 64)):
                    h = 2 * ot + hi
                    mid = l0 + 128 if (diag and l0 + 128 < CW) else CW
                    nc.tensor.matmul(
                        otps[hb][:, l0:mid],
                        VA[:, j, h * 65:(h + 1) * 65],
                        et3[:, hi, l0:mid],
                        start=(j == 0), stop=(j == j_hi - 1))
                    if mid < CW:
                        nc.tensor.matmul(
                            otps[hb][:, mid:CW],
                            VA[:, j, h * 65:(h + 1) * 65],
                            et3[:, hi, mid:CW],
                            start=(j == 0), stop=(j == j_hi - 1))

            pending = []
            for j in range(j_hi):      # s2 blocks of 128
                l0 = max(0, 128 * j - cw_lo)
                et3 = epn.tile([128, 2, CW], bf16, tag="e", name="et")
                sc_ps = scn.tile([128, 2, CW], f32, tag="sc", name="sc_ps")
                for hi, hb in enumerate((0, 64)):
                    nc.tensor.matmul(
                        sc_ps[:, hi, l0:CW],
                        KT[hb:hb + 64, ot, j * 128:(j + 1) * 128],
                        QT[hb:hb + 64, ot, cw_lo + l0:cw_lo + CW],
                        start=True, stop=True)
                nc.scalar.activation(
                    et3[:, :, l0:CW], sc_ps[:, :, l0:CW],
                    mybir.ActivationFunctionType.Exp,
                    scale=1.0 / math.sqrt(HEAD_DIM))
                dl = 128 * j - cw_lo
                if dl >= 0:
                    nc.vector.tensor_mul(
                        et3[:, :, dl:dl + 128],
                        et3[:, :, dl:dl + 128], dm_sb[:])
                pending.append((j, l0, et3))
                if len(pending) > 2:
                    emit_pv(*pending.pop(0))
            for p in pending:
                emit_pv(*p)
            for hb in (0, 64):
                rec = rp.tile([1, CW], f32, tag="rec", name="rec")
                nc.vector.reciprocal(rec[:], otps[hb][64:65, :])
                bc = bp.tile([64, CW], f32, tag="bc", name="bc")
                nc.gpsimd.partition_broadcast(bc[:], rec[:])
                nc.vector.tensor_mul(
                    OT[hb:hb + 64, ot, cw_lo:cw_lo + CW],
                    otps[hb][0:64, :], bc[:])

        def proj_blocks(sbs):
            # ---- output projection, one 128-row s1 block per sb.
            # pj halves come from ps1 (idle post-qk) so projection never
            # contends with attention scores for the scn banks.
            for sb in sbs:
                st = stg.tile([128, DIM], f32, tag="st", name="st")
                for half in range(2):
                    pj = ps1.tile([128, 512], f32, tag="ps1", name="pj")
                    for kt in range(4):
                        nc.tensor.matmul(
                            pj[:],
                            OT[:, kt, sb * 128:(sb + 1) * 128],
                            WO[:, kt, half * 512:(half + 1) * 512],
                            start=(kt == 0), stop=(kt == 3))
                    nc.vector.tensor_copy(
                        st[:, half * 512:(half + 1) * 512], pj[:])
                nc.sync.dma_start(
                    out.ap()[sb * 128:(sb + 1) * 128, :], st[:])

        def qk_ot(ot, scs=(0, 1, 2, 3)):
            for sc in scs:
                sl = slice(sc * 512, (sc + 1) * 512)
                for wsb, dest in ((WK, KT), (WQ, QT)):
                    acc = ps1.tile([128, 512], f32, tag="ps1", name="acc")
                    for kt in range(8):
                        nc.tensor.matmul(
                            acc[:],
                            wsb[:, kt, ot * 128:(ot + 1) * 128],
                            XS[:, kt, sl],
                            start=(kt == 0), stop=(kt == 7))
                    raw = rawp.tile([128, 512], bf16, tag="raw", name="raw")
                    nc.vector.tensor_copy(raw[:], acc[:])
                    rot = ps1.tile([128, 512], f32, tag="ps1", name="rot")
                    nc.tensor.matmul(rot[:], rt_sb[:], raw[:],
                                     start=True, stop=True)
                    t1 = tp.tile([128, 512], bf16, tag="t", name="t1")
                    nc.vector.tensor_mul(t1[:], raw[:], cos_sb[:, sl])
                    t2 = tp.tile([128, 512], bf16, tag="t", name="t2")
                    nc.vector.tensor_mul(t2[:], rot[:], sin_sb[:, sl])
                    nc.vector.tensor_add(dest[:, ot, sl], t1[:], t2[:])

        # --- input DMAs.  x + K/Q weights on sync (x first so the v
        # pipeline starts immediately); V weights on scalar; tables on
        # gpsimd (software DGE, separate path).
        nc.sync.dma_start(XS[:, :, 0:256], xT3[:, :, 0:256])
        for o in range(8):
            nc.sync.dma_start(WV[:, o, :], wv3[:, o, :])
        for sc2 in range(1, 8):
            sl = slice(sc2 * 256, (sc2 + 1) * 256)
            nc.sync.dma_start(XS[:, :, sl], xT3[:, :, sl])
        for o in range(8):
            nc.sync.dma_start(WK[:, o, :], wk3[:, o, :])
        for o in range(8):
            nc.sync.dma_start(WQ[:, o, :], wq3[:, o, :])
        nc.gpsimd.dma_start(rt_sb[:], rT.ap())
        nc.gpsimd.dma_start(cos_sb[:], cosT.ap())
        nc.gpsimd.dma_start(sin_sb[:], sinT.ap())
        nc.gpsimd.dma_start(dm_sb[:], dmask.ap())
        nc.gpsimd.dma_start(WO[:], wo3)

        # ones column of VA (written once; disjoint from V copies)
        va4 = VA[:].rearrange("p t (h c) -> p t h c", c=65)
        nc.vector.tensor_scalar(
            va4[:, :, :, 64:65], va4[:, :, :, 64:65],
            0.0, 1.0, mybir.AluOpType.mult, mybir.AluOpType.add)

        # --- V projection (s2 blocks of 128; tail blocks deferred into ---
        # --- the post-qk region as PE-dense filler)                     ---
        def v_blocks(s2ts, copy_eng):
            for s2t in s2ts:
                acc = ps1.tile([128, O], f32, tag="ps1", name="vacc")
                for kt in range(8):
                    nc.tensor.matmul(
                        acc[:],
                        XS[:, kt, s2t * 128:(s2t + 1) * 128],
                        WV[:, kt, :],
                        start=(kt == 0), stop=(kt == 7))
                vsl = VA[:, s2t, :].rearrange("p (h c) -> p h c", c=65)
                copy_eng(
                    vsl[:, :, 0:64],
                    acc[:].rearrange("p (h c) -> p h c", c=64))

        v_blocks(range(12), nc.scalar.copy)

        # --- interleave: PE-dense qk/proj segments alternate with ---
        # --- ACT-dense attention chunks                            ---
        qk_ot(0)
        attn_chunk(0, 0 * CW)
        qk_ot(1)
        attn_chunk(0, 1 * CW)
        attn_chunk(1, 0 * CW)
        qk_ot(2)
        attn_chunk(0, 2 * CW)
        attn_chunk(1, 1 * CW)
        attn_chunk(2, 0 * CW)
        qk_ot(3)
        attn_chunk(1, 2 * CW)
        v_blocks((12, 13), nc.vector.tensor_copy)
        attn_chunk(2, 1 * CW)
        v_blocks((14, 15), nc.vector.tensor_copy)
        attn_chunk(0, 3 * CW)
        attn_chunk(3, 0 * CW)
        attn_chunk(1, 3 * CW)
        proj_blocks([0, 1])
        attn_chunk(2, 2 * CW)
        proj_blocks([2, 3])
        attn_chunk(3, 1 * CW)
        attn_chunk(2, 3 * CW)
        proj_blocks([4, 5, 6, 7])
        attn_chunk(3, 2 * CW)
        attn_chunk(3, 3 * CW)
        proj_blocks([8, 9, 10, 11])
        proj_blocks([12, 13, 14, 15])


def _get_nc(reps=1):
    if reps not in _NC:
        _NC[reps] = _build_nc(reps)
    return _NC[reps]


def make_in_maps(x, Wq, Wk, Wv, Wo):
    import ml_dtypes
    bf16 = ml_dtypes.bfloat16
    cosT, sinT = _rope_tables()
    rT = _rot_matrix()
    # keep where s2 <= s1 in (s2, s1) indexing -> upper-tri incl diag,
    # duplicated for the merged head-pair mul
    dm1 = np.triu(np.ones((128, 128), dtype=np.float32))
    dm = np.stack([dm1, dm1], axis=1).astype(bf16)      # (128, 2, 128)
    in_maps = []
    for c in range(N_CORES):
        b, g = c // 2, c % 2
        rows = slice(g * O, (g + 1) * O)
        in_maps.append({
            "xT": np.ascontiguousarray(x[b].T).astype(bf16),
            "wqT": np.ascontiguousarray(Wq[rows, :].T).astype(bf16),
            "wkT": np.ascontiguousarray(Wk[rows, :].T).astype(bf16),
            "wvT": np.ascontiguousarray(Wv[rows, :].T).astype(bf16),
            "woT": np.ascontiguousarray(Wo[:, rows].T).astype(bf16),
            "cosT": cosT.astype(bf16), "sinT": sinT.astype(bf16),
            "rT": rT.astype(bf16), "dmask": dm,
        })
    return in_maps


def _numpy_fallback(x, Wq, Wk, Wv, Wo, mask):
    cosT, sinT = _rope_tables()
    cos, sin = cosT[:64].T, sinT[:64].T                      # (S, 64)
    xq = x @ Wq.T
    xk = x @ Wk.T
    xv = x @ Wv.T

    def heads(t):
        return t.reshape(B, S, NUM_HEADS, HEAD_DIM).transpose(0, 2, 1, 3)

    q, k, v = heads(xq), heads(xk), heads(xv)

    def rot(t):
        return np.concatenate([-t[..., 32:], t[..., :32]], axis=-1)

    q = q * cos + rot(q) * sin
    k = k * cos + rot(k) * sin
    sc = np.einsum("bhsd,bhtd->bhst", q, k) / math.sqrt(HEAD_DIM)
    sc = np.where(mask[None, None] == 0, -np.inf, sc)
    sc = sc - sc.max(axis=-1, keepdims=True)
    e = np.exp(sc)
    p = e / e.sum(axis=-1, keepdims=True)
    o = np.einsum("bhst,bhtd->bhsd", p, v)
    o = o.transpose(0, 2, 1, 3).reshape(B, S, DIM)
    return (o @ Wo.T).astype(np.float32)


def kernel(x, Wq, Wk, Wv, Wo, mask):
    x = np.asarray(x)
    mask = np.asarray(mask)
    causal = bool(
        np.array_equal(np.asarray(mask, dtype=np.int64),
                       np.tril(np.ones((S, S), dtype=np.int64))))
    if not causal:
        return _numpy_fallback(
            np.asarray(x, np.float32), np.asarray(Wq, np.float32),
            np.asarray(Wk, np.float32), np.asarray(Wv, np.float32),
            np.asarray(Wo, np.float32), mask)

    from concourse.bass_utils import run_bass_kernel_spmd

    nc = _get_nc()
    in_maps = make_in_maps(x, Wq, Wk, Wv, Wo)
    res = run_bass_kernel_spmd(nc, in_maps, list(range(N_CORES)))
    out = np.empty((B, S, DIM), dtype=np.float32)
    for b in range(B):
        out[b] = res.results[2 * b]["out"] + res.results[2 * b + 1]["out"]
    return out
